# revision 4
# baseline (speedup 1.0000x reference)
"""BertSelfAttention (with segment-embedding score bias) on 8 trn2 NeuronCores.

Math implemented (reference semantics):
    q = X @ Wq.T + bq ; k = X @ Wk.T ; v = X @ Wv.T + bv      (per head h)
    scores = (q*s) @ k.T + (q + b_q_s) @ segrep.T + mask ;  s = 1/sqrt(DH)
    out = softmax(scores) @ v

Key algebraic folds (exact):
    (q*s) @ (k + segrep/s).T = (q*s) @ k.T + q @ segrep.T
    remaining term (b_q_s @ segrep.T + mask) is query-independent ->
    a per-key additive bias applied inside the exp() activation.
    segrep = seg_table[seg_ids] is a 2-row gather -> one K=2 matmul with
    one-hot(seg_ids) rows appended to the K'-projection accumulation.
    Softmax denominator = ones-column appended to V in the PV matmul.

Sharding: tensor-parallel over heads; core c owns heads 2c, 2c+1.
Each core reads the full tokens, computes its head-slice of Q/K'/V and its
slice of the output; host concatenates along the hidden dim. No collectives.
"""

import os
import sys

for _p in ("/opt/trn_rl_repo", "/root/.axon_site/_ro/trn_rl_repo"):
    if os.path.isdir(_p) and _p not in sys.path:
        sys.path.append(_p)

import numpy as np
import ml_dtypes

B, S, NH, DH = 4, 2048, 16, 64
HID = NH * DH          # 1024
T = B * S              # 8192
N_CORES = 8
HPC = NH // N_CORES    # heads per core = 2
DPC = HPC * DH         # out dims per core = 128
SCALE = 1.0 / 8.0      # 1/sqrt(DH)
KT = HID // 128        # 8 contraction tiles
CHUNK = 1024           # token chunk for projections
NCH = T // CHUNK       # 8
SKT = S // 128         # 16 key tiles per sequence
QH = 2                 # query halves per sequence
QBLK = S // QH         # 1024
NQT = QBLK // 128      # 8 query tiles per half

_cache = {}


def _build_program():
    import concourse.bacc as bacc
    import concourse.tile as tile
    from concourse import masks, mybir
    from contextlib import ExitStack

    bf16 = mybir.dt.bfloat16
    f32 = mybir.dt.float32
    f32r = mybir.dt.float32r
    Exp = mybir.ActivationFunctionType.Exp

    nc = bacc.Bacc("TRN2", target_bir_lowering=False, debug=False,
                   num_devices=N_CORES)
    xb = nc.dram_tensor("xb", [T, HID], bf16, kind="ExternalInput")
    wq = nc.dram_tensor("wq", [HID, DPC], bf16, kind="ExternalInput")
    wk = nc.dram_tensor("wk", [HID, DPC], bf16, kind="ExternalInput")
    wv = nc.dram_tensor("wv", [HID, DPC], bf16, kind="ExternalInput")
    segw = nc.dram_tensor("segw", [2, DPC], bf16, kind="ExternalInput")
    oh = nc.dram_tensor("oh", [2, T], bf16, kind="ExternalInput")
    rb = nc.dram_tensor("rb", [128, 128], f32, kind="ExternalInput")
    bq = nc.dram_tensor("bq", [DPC, 1], f32, kind="ExternalInput")
    bv = nc.dram_tensor("bv", [DPC, 1], f32, kind="ExternalInput")
    outd = nc.dram_tensor("out", [T, DPC], f32, kind="ExternalOutput")

    with tile.TileContext(nc) as tc, ExitStack() as octx:
        const = octx.enter_context(tc.tile_pool(name="const", bufs=1))
        res = octx.enter_context(tc.tile_pool(name="res", bufs=1))

        # resident activations (partition dim = 2 heads x 64 dims)
        qT = res.tile([128, T], f32r)                 # Q^T, pre-scaled, +bias
        kT = res.tile([128, T], f32r)                 # K'^T (seg folded in)
        vsb = res.tile([128, (T // 128) * 130], bf16)  # [V_h0|1|V_h1|1] per tile

        rb_sb = const.tile([128, 128], f32)
        bq_sb = const.tile([DPC, 1], f32)
        bv_sb = const.tile([DPC, 1], f32)
        ident = const.tile([128, 128], bf16)
        nc.sync.dma_start(rb_sb[:], rb[:])
        nc.sync.dma_start(bq_sb[:], bq[:])
        nc.sync.dma_start(bv_sb[:], bv[:])
        masks.make_identity(nc, ident[:])
        nc.vector.memset(vsb[:], 1.0)   # preset ones cols; data cols overwritten

        # ---------------- Phase 1: projections ----------------
        with ExitStack() as p1:
            p1c = p1.enter_context(tc.tile_pool(name="p1c", bufs=1))
            xt_pool = p1.enter_context(tc.tile_pool(name="xt", bufs=2 * KT))
            vt_pool = p1.enter_context(tc.tile_pool(name="vt", bufs=2))
            ppsum = p1.enter_context(
                tc.tile_pool(name="ppsum", bufs=3, space="PSUM"))
            vtpsum = p1.enter_context(
                tc.tile_pool(name="vtpsum", bufs=2, space="PSUM"))

            wq_sb = p1c.tile([128, KT, DPC], bf16)
            wk_sb = p1c.tile([128, KT, DPC], bf16)
            wv_sb = p1c.tile([128, KT, DPC], bf16)
            segw_sb = p1c.tile([2, DPC], bf16)
            oh_sb = p1c.tile([2, T], bf16)
            nc.sync.dma_start(wq_sb[:], wq.rearrange("(kt p) d -> p kt d", p=128))
            nc.sync.dma_start(wk_sb[:], wk.rearrange("(kt p) d -> p kt d", p=128))
            nc.sync.dma_start(wv_sb[:], wv.rearrange("(kt p) d -> p kt d", p=128))
            nc.sync.dma_start(segw_sb[:], segw[:])
            nc.sync.dma_start(oh_sb[:], oh[:])

            for ci in range(NCH):
                cs = slice(ci * CHUNK, (ci + 1) * CHUNK)
                xts = []
                for kt in range(KT):
                    xt = xt_pool.tile([128, CHUNK], bf16, tag="xt")
                    nc.sync.dma_start(
                        xt[:], xb[cs, kt * 128:(kt + 1) * 128], transpose=True)
                    xts.append(xt)

                # PSUM bank limit: one matmul's fp32 output <= 512 cols,
                # bank-aligned -> emit per-512 column groups.
                def proj(psum_tile, w_sb, seg=False):
                    for nn in range(CHUNK // 512):
                        o = psum_tile[:, nn * 512:(nn + 1) * 512]
                        for kt in range(KT):
                            nc.tensor.matmul(
                                o, w_sb[:, kt, :],
                                xts[kt][:, nn * 512:(nn + 1) * 512],
                                start=(kt == 0),
                                stop=(kt == KT - 1) and not seg)
                        if seg:
                            nc.tensor.matmul(
                                o, segw_sb[:],
                                oh_sb[:, ci * CHUNK + nn * 512:
                                      ci * CHUNK + (nn + 1) * 512],
                                start=False, stop=True)

                qp = ppsum.tile([128, CHUNK], f32, tag="pp")
                proj(qp, wq_sb)
                nc.vector.tensor_scalar_add(qT[:, cs], qp[:], bq_sb[:, 0:1])

                kp = ppsum.tile([128, CHUNK], f32, tag="pp")
                proj(kp, wk_sb, seg=True)
                nc.vector.tensor_copy(kT[:, cs], kp[:])

                vp = ppsum.tile([128, CHUNK], f32, tag="pp")
                proj(vp, wv_sb)
                vt = vt_pool.tile([128, CHUNK], bf16, tag="vt")
                nc.vector.tensor_scalar_add(vt[:], vp[:], bv_sb[:, 0:1])
                for tt in range(CHUNK // 128):
                    gt = ci * (CHUNK // 128) + tt
                    vtp = vtpsum.tile([128, 128], bf16, tag="vtp")
                    nc.tensor.transpose(
                        vtp[:], vt[:, tt * 128:(tt + 1) * 128], ident[:])
                    nc.vector.tensor_copy(
                        vsb[:, gt * 130:(gt + 1) * 130]
                        .rearrange("p (h x) -> p h x", h=2)[:, :, 0:64],
                        vtp[:].rearrange("p (h d) -> p h d", h=2))

        # ---------------- Phase 2: attention ----------------
        # Per (batch, head, query-half): compute exp(scores^T) for all 16 key
        # tiles into resident SBUF tiles, then accumulate P^T@[V|1] per query
        # tile (one PSUM accumulation group per bank at a time).
        with ExitStack() as p2:
            pt_pool = p2.enter_context(tc.tile_pool(name="pt", bufs=SKT + 4))
            stage_pool = p2.enter_context(tc.tile_pool(name="stage", bufs=2))
            rcp_pool = p2.enter_context(tc.tile_pool(name="rcp", bufs=8))
            sc_psum = p2.enter_context(
                tc.tile_pool(name="scp", bufs=2, space="PSUM"))
            ctx_psum = p2.enter_context(
                tc.tile_pool(name="ctxp", bufs=3, space="PSUM"))

            for b in range(B):
                stage = stage_pool.tile([128, 16 * 128], f32, tag="stage")
                for hl in range(HPC):
                    pb = hl * 64
                    for qh in range(QH):
                        q0 = b * S + qh * QBLK
                        pts = []
                        for kt in range(SKT):
                            k0 = b * S + kt * 128
                            sp = sc_psum.tile([128, QBLK], f32, tag="sc")
                            ksl = kT[pb:pb + 64, k0:k0 + 128]
                            for nn in range(QBLK // 512):
                                qsl = qT[pb:pb + 64,
                                         q0 + nn * 512:q0 + (nn + 1) * 512]
                                nc.tensor.matmul(
                                    sp[:, nn * 512:(nn + 1) * 512],
                                    ksl, qsl,
                                    start=True, stop=True)
                            pt = pt_pool.tile([128, QBLK], bf16, tag="pt")
                            col = hl * 64 + b * 16 + kt
                            nc.scalar.activation(
                                pt[:], sp[:], Exp,
                                bias=rb_sb[:, col:col + 1], scale=1.0)
                            pts.append(pt)
                        for qt in range(NQT):
                            ctxp = ctx_psum.tile([128, 128], f32, tag="ctx")
                            for kt in range(SKT):
                                vb = (b * 16 + kt) * 130 + hl * 65
                                nc.tensor.matmul(
                                    ctxp[:, 0:65],
                                    pts[kt][:, qt * 128:(qt + 1) * 128],
                                    vsb[:, vb:vb + 65],
                                    start=(kt == 0), stop=(kt == SKT - 1))
                            gq = qh * NQT + qt
                            rcp = rcp_pool.tile([128, 1], f32, tag="rcp")
                            nc.vector.reciprocal(rcp[:], ctxp[:, 64:65])
                            nc.vector.tensor_scalar_mul(
                                stage[:, gq * 128 + pb:gq * 128 + pb + 64],
                                ctxp[:, 0:64], rcp[:, 0:1])
                nc.sync.dma_start(
                    outd[b * S:(b + 1) * S, :]
                    .rearrange("(gq q) hd -> q gq hd", q=128),
                    stage[:].rearrange("q (gq hd) -> q gq hd", hd=DPC))

    nc.compile()
    return nc


def get_program():
    if "nc" not in _cache:
        _cache["nc"] = _build_program()
    return _cache["nc"]


def make_in_maps(hidden_states, attention_mask, seg_ids, Wq, bq, Wk, Wv, bv,
                 seg_table, b_q_s):
    """Host-side shard + layout prep. Cheap (weights/bias reshapes + one
    bf16 cast of X); all O(T*S) math stays on device."""
    bf = ml_dtypes.bfloat16
    X = np.asarray(hidden_states, np.float32).reshape(T, HID)
    xb = np.ascontiguousarray(X.astype(bf))
    m = np.asarray(seg_ids).reshape(T).astype(np.int64)
    oh = np.zeros((2, T), bf)
    oh[0, :] = (m == 0).astype(bf)
    oh[1, :] = (m == 1).astype(bf)
    mask = np.asarray(attention_mask, np.float32).reshape(B, S)
    st = np.asarray(seg_table, np.float32)              # [2, HID]
    bqs = np.asarray(b_q_s, np.float32).reshape(NH, DH)
    Wq = np.asarray(Wq, np.float32)
    Wk = np.asarray(Wk, np.float32)
    Wv = np.asarray(Wv, np.float32)
    bq = np.asarray(bq, np.float32)
    bv = np.asarray(bv, np.float32)

    in_maps = []
    for c in range(N_CORES):
        sl = slice(c * DPC, (c + 1) * DPC)
        rb_c = np.zeros((128, 128), np.float32)
        for hl in range(HPC):
            h = c * HPC + hl
            c01 = st[:, h * DH:(h + 1) * DH] @ bqs[h]   # [2]
            val = c01[m.reshape(B, S)] + mask           # [B, S]
            rb_c[:, hl * 64:(hl + 1) * 64] = (
                val.reshape(B, 16, 128).transpose(2, 0, 1).reshape(128, 64))
        in_maps.append({
            "xb": xb,
            "wq": np.ascontiguousarray((Wq[sl, :] * SCALE).T).astype(bf),
            "wk": np.ascontiguousarray(Wk[sl, :].T).astype(bf),
            "wv": np.ascontiguousarray(Wv[sl, :].T).astype(bf),
            "segw": np.ascontiguousarray(st[:, sl] / SCALE).astype(bf),
            "oh": oh,
            "rb": rb_c,
            "bq": np.ascontiguousarray((bq[sl] * SCALE).reshape(DPC, 1)),
            "bv": np.ascontiguousarray(bv[sl].reshape(DPC, 1)),
        })
    return in_maps


def assemble_output(results):
    return np.concatenate(
        [np.asarray(r["out"], np.float32).reshape(B, S, DPC) for r in results],
        axis=2)


def kernel(hidden_states, attention_mask, seg_ids, Wq, bq, Wk, Wv, bv,
           seg_table, b_q_s):
    from concourse.bass_utils import run_bass_kernel_spmd
    nc = get_program()
    in_maps = make_in_maps(hidden_states, attention_mask, seg_ids, Wq, bq,
                           Wk, Wv, bv, seg_table, b_q_s)
    res = run_bass_kernel_spmd(nc, in_maps, list(range(N_CORES)))
    return assemble_output(res.results)


if __name__ == "__main__":
    nc = get_program()
    print("program built + compiled ok;",
          len(nc.m.functions[0].basic_blocks[0].instructions)
          if hasattr(nc.m.functions[0], "basic_blocks") else "")
